# revision 35
# baseline (speedup 1.0000x reference)
"""Causal Performer (FAVOR+) Trainium2 kernel.

Sharding: 8 cores = 2 (batch) x 4 (head groups of 4 heads).  Each core
computes its 4 heads for one batch and returns a partial [4096, 2048]
output (its heads' contribution through w_o) in bf16; the host sums
the 4 partials per batch in f32.

Structure: software-pipelined over 512-position sequence blocks.  Each
iteration issues SCAN(k-1) (sparse, DVE-gated small matmuls) BEFORE
PROJ(k) (dense N=512 matmuls with no compute deps), so the scheduler
fills every scan stall with projection work and the PE activity never
dips low enough for the HAM clock gate to re-throttle.

  - Packed-head layout: the 4 heads' 8 random features live at
    partition offsets 32h of one [128, 512] tile; exp/transpose/reduce/
    normalize are whole-tile ops, and the small matmuls use 32-aligned
    tile_position packing (den: M=1 col groups, su: M=8 col groups,
    A^T/Z: K=8 row groups).
  - Numerator accumulated in short-lived [128,256] PSUM chunks per
    (head, half), divided by the broadcast reciprocal as produced.
  - den/su accumulate interleaved per-head chains into memset-zeroed
    banks with start=False (a start=True whole-bank has_written clear
    would corrupt sibling heads).
  - Output partials in bf16; host sums in f32.
"""

import os
import numpy as np
import ml_dtypes

from concourse import bacc, mybir
import concourse.tile as tile
from concourse.bass import ts
from concourse.bass_utils import run_bass_kernel_spmd
from concourse.masks import make_identity

B, S, D = 2, 4096, 2048
H_PER = 4            # heads per core
DK = 128
NB = 8
SBLK = 512           # sequence block
NBLK = S // SBLK     # 8
NSUB = SBLK // 128   # 4 sub-chunks of 128
NCH = 2              # numerator chunks per head (width 256)
CW = SBLK // NCH     # 256
EPS = 1e-6

bf16 = mybir.dt.bfloat16
f32 = mybir.dt.float32

LAST_EXEC_TIME_NS = None
_CACHE = {}


def _build():
    nc = bacc.Bacc("TRN2", target_bir_lowering=False, debug=False)

    xq_d = nc.dram_tensor("xq", [D, S], bf16, kind="ExternalInput").ap()
    xk_d = nc.dram_tensor("xk", [D, S], bf16, kind="ExternalInput").ap()
    xv_d = nc.dram_tensor("xv", [D, S], bf16, kind="ExternalInput").ap()
    wqom_d = nc.dram_tensor("wqom", [D, 128], bf16, kind="ExternalInput").ap()
    wkom_d = nc.dram_tensor("wkom", [D, 128], bf16, kind="ExternalInput").ap()
    wv_d = nc.dram_tensor("wv", [D, 512], bf16, kind="ExternalInput").ap()
    wo_d = nc.dram_tensor("wo", [512, D], bf16, kind="ExternalInput").ap()
    mask_d = nc.dram_tensor("mask", [128, SBLK], f32, kind="ExternalInput").ap()
    part_d = nc.dram_tensor("part", [S, D], bf16, kind="ExternalOutput").ap()

    KC = D // 128
    Exp = mybir.ActivationFunctionType.Exp

    with tile.TileContext(nc) as tc:
        with tc.tile_pool(name="const", bufs=1) as const, \
             tc.tile_pool(name="wpool", bufs=1) as wpool, \
             tc.tile_pool(name="state", bufs=1) as state, \
             tc.tile_pool(name="xpool", bufs=2) as xpool, \
             tc.tile_pool(name="vpool", bufs=2) as vpool, \
             tc.tile_pool(name="featpool", bufs=2) as featpool, \
             tc.tile_pool(name="atmpool", bufs=1) as atmpool, \
             tc.tile_pool(name="otpool", bufs=6) as otpool, \
             tc.tile_pool(name="bcbpool", bufs=4) as bcbpool, \
             tc.tile_pool(name="osbpool", bufs=2) as osbpool, \
             tc.tile_pool(name="miscpool", bufs=2) as miscpool, \
             tc.tile_pool(name="scrpool", bufs=1) as scrpool, \
             tc.tile_pool(name="psqk", bufs=2, space="PSUM") as psqk, \
             tc.tile_pool(name="psv", bufs=2, space="PSUM") as psv, \
             tc.tile_pool(name="psden", bufs=1, space="PSUM") as psden, \
             tc.tile_pool(name="psmisc", bufs=3, space="PSUM") as psmisc:

            ident = const.tile([128, 128], bf16, name="ident")
            make_identity(nc, ident)
            mask_sb = const.tile([128, SBLK], f32, name="mask_sb")
            nc.sync.dma_start(mask_sb[:], mask_d[:])
            ones_col = const.tile([128, 1], bf16, name="ones_col")
            nc.vector.memset(ones_col[:], 1.0)
            ones_row = const.tile([97, 128], bf16, name="ones_row")
            nc.vector.memset(ones_row[:], 1.0)

            wqom_sb = wpool.tile([128, KC, 128], bf16, name="wqom_sb")
            nc.sync.dma_start(wqom_sb[:], wqom_d.rearrange("(c p) m -> p c m", p=128))
            wkom_sb = wpool.tile([128, KC, 128], bf16, name="wkom_sb")
            nc.sync.dma_start(wkom_sb[:], wkom_d.rearrange("(c p) m -> p c m", p=128))
            # wv/wo DMAs issued after block 0's x loads (cold-start order)
            wv_sb = wpool.tile([128, KC, 512], bf16, name="wv_sb")
            wo_sb = wpool.tile([128, H_PER, D], bf16, name="wo_sb")

            # persistent scan state, head h at partitions 32h..32h+8:
            # cols 0:128 = Z, col 128 = z
            Zsb = state.tile([128, 132], f32, name="Zsb")
            nc.vector.memset(Zsb[:], 0.0)
            Zb16 = state.tile([128, 132], bf16, name="Zb16")
            nc.vector.memset(Zb16[:], 0.0)

            def proj(blk):
                """Projections for block blk: returns tiles dict."""
                s0 = blk * SBLK
                xq_sb = xpool.tile([128, KC, SBLK], bf16, name=f"xq{blk}", tag="xq")
                nc.sync.dma_start(
                    xq_sb[:],
                    xq_d.rearrange("(c p) s -> p c s", p=128)[:, :, s0:s0 + SBLK])
                xk_sb = xpool.tile([128, KC, SBLK], bf16, name=f"xk{blk}", tag="xk")
                nc.sync.dma_start(
                    xk_sb[:],
                    xk_d.rearrange("(c p) s -> p c s", p=128)[:, :, s0:s0 + SBLK])
                xv_sb = xpool.tile([128, KC, SBLK], bf16, name=f"xv{blk}", tag="xv")
                nc.sync.dma_start(
                    xv_sb[:],
                    xv_d.rearrange("(c p) s -> p c s", p=128)[:, :, s0:s0 + SBLK])
                if blk == 0:
                    nc.sync.dma_start(wv_sb[:], wv_d.rearrange("(c p) m -> p c m", p=128))
                    nc.sync.dma_start(wo_sb[:], wo_d.rearrange("(c p) m -> p c m", p=128))

                # fused q/k feature projections: head h rows at 32h
                qf_p = psqk.tile([128, SBLK], f32, name=f"qfp{blk}", tag="qk")
                kf_p = psqk.tile([128, SBLK], f32, name=f"kfp{blk}", tag="qk")
                for dst, wsb, xsb in ((qf_p, wqom_sb, xq_sb), (kf_p, wkom_sb, xk_sb)):
                    for kc in range(KC):
                        nc.tensor.matmul(dst[:], wsb[:, kc, :], xsb[:, kc, :],
                                         start=(kc == 0), stop=(kc == KC - 1))
                qsq = scrpool.tile([128, SBLK], f32, name=f"qsq{blk}", tag="qsq")
                nc.scalar.square(qsq[:], qf_p[:])
                ksq = scrpool.tile([128, SBLK], f32, name=f"ksq{blk}", tag="ksq")
                nc.scalar.square(ksq[:], kf_p[:])
                qfe = featpool.tile([128, SBLK], bf16, name=f"qfe{blk}", tag="qfe")
                nc.scalar.activation(qfe[:], qsq[:], Exp, scale=-0.5)
                kfe = featpool.tile([128, SBLK], bf16, name=f"kfe{blk}", tag="kfe")
                nc.scalar.activation(kfe[:], ksq[:], Exp, scale=-0.5)

                # k features transposed to [s, f-packed]
                kfs = featpool.tile([128, NSUB, 128], bf16, name=f"kfs{blk}", tag="kfs")
                for j in range(NSUB):
                    kT_p = psmisc.tile([128, 128], bf16, name=f"kT{blk}_{j}", tag="m")
                    nc.tensor.transpose(kT_p[:], kfe[:, ts(j, 128)], ident[:])
                    nc.vector.tensor_copy(kfs[:, j, :], kT_p[:])

                # normalizers: ksum[s, j*4+h] over the 8 real features
                ksum = miscpool.tile([128, NSUB * H_PER], f32, name=f"ksum{blk}", tag="ksum")
                nc.vector.reduce_sum(
                    ksum[:].rearrange("p (a b) -> p a b", b=H_PER),
                    kfs[:].rearrange("p a (b c) -> p a b c", c=32)[:, :, :, 0:NB],
                    axis=mybir.AxisListType.X)
                nc.vector.tensor_scalar_add(ksum[:], ksum[:], EPS)
                krec = miscpool.tile([128, NSUB * H_PER], f32, name=f"krec{blk}", tag="krec")
                nc.vector.reciprocal(krec[:], ksum[:])

                # normalized k features for the state update
                kfn = featpool.tile([128, NSUB, 128], bf16, name=f"kfn{blk}", tag="kfn")
                for j in range(NSUB):
                    for h in range(H_PER):
                        nc.vector.tensor_scalar(
                            out=kfn[:, j, 32 * h:32 * h + 32],
                            in0=kfs[:, j, 32 * h:32 * h + 32],
                            scalar1=krec[:, 4 * j + h:4 * j + h + 1], scalar2=None,
                            op0=mybir.AluOpType.mult)

                # v projection: vha [s_sub(128), j, head, 132] (+ones col)
                vha = vpool.tile([128, NSUB, H_PER, 132], bf16, name=f"vha{blk}", tag="vha")
                for j in range(NSUB):
                    pp = psv.tile([128, SBLK], f32, name=f"pv{blk}_{j}", tag="pp")
                    for kc in range(KC):
                        nc.tensor.matmul(pp[:], xv_sb[:, kc, ts(j, 128)],
                                         wv_sb[:, kc, :],
                                         start=(kc == 0), stop=(kc == KC - 1))
                    nc.scalar.copy(vha[:, j, :, 0:128],
                                   pp.rearrange("p (h d) -> p h d", d=128))
                    nc.vector.memset(vha[:, j, :, 128:129], 1.0)

                return dict(qfe=qfe, kfe=kfe, kfn=kfn, krec=krec, vha=vha)

            def scan_oproj(blk, t):
                """Scan + output projection for block blk using tiles t."""
                s0 = blk * SBLK
                qfe, kfe, kfn, krec, vha = (t[k] for k in
                                            ("qfe", "kfe", "kfn", "krec", "vha"))

                # masked A^T blocks (row-group packed), scale+mask on DVE
                atm = {}
                for i2 in range(NSUB):
                    n_i = SBLK - 128 * i2
                    atp = []
                    for h in range(H_PER):
                        at_p = psmisc.tile([128, SBLK], f32, name=f"at{blk}_{i2}_{h}", tag="m")
                        nc.tensor.matmul(at_p[:, :n_i], kfe[32 * h:32 * h + NB, ts(i2, 128)],
                                         qfe[32 * h:32 * h + NB, 128 * i2:SBLK],
                                         start=True, stop=True,
                                         tile_position=(32 * h, 0))
                        atp.append(at_p)
                    for h in range(H_PER):
                        am = atmpool.tile([128, n_i], bf16, name=f"am{blk}_{i2}_{h}",
                                          tag=f"atm{i2}", bufs=4)
                        nc.vector.scalar_tensor_tensor(
                            out=am[:], in0=atp[h][:, :n_i],
                            scalar=krec[:, 4 * i2 + h:4 * i2 + h + 1],
                            in1=mask_sb[:, :n_i],
                            op0=mybir.AluOpType.mult, op1=mybir.AluOpType.mult)
                        atm[(i2, h)] = am

                # denominators, head h at partition 32h (col-group packed);
                # memset + start=False: interleaved per-head chains must
                # accumulate onto zeroed data (start would clear bank-wide).
                den_all = psden.tile([128, SBLK], f32, name=f"den{blk}", tag="den")
                nc.vector.memset(den_all[:], 0.0)
                for i2 in range(NSUB):
                    for h in range(H_PER):
                        nc.tensor.matmul(den_all[32 * h:32 * h + 1, 128 * i2:SBLK],
                                         ones_col[:], atm[(i2, h)][:],
                                         start=False, stop=False,
                                         tile_position=(0, 32 * h),
                                         skip_group_check=True)
                for h in range(H_PER):
                    nc.tensor.matmul(den_all[32 * h:32 * h + 1, :],
                                     Zb16[32 * h:32 * h + NB, 128:129],
                                     qfe[32 * h:32 * h + NB, :],
                                     start=False, stop=(h == H_PER - 1),
                                     tile_position=(32 * h, 32 * h),
                                     skip_group_check=True)

                # reciprocal of denominators, pipelined in 128-col pieces
                # (a single [97,512] reciprocal is ~3.4us and head-of-line
                # blocks the bc matmuls in the PE stream)
                drf = scrpool.tile([97, SBLK], f32, name=f"drf{blk}", tag="drf")
                drb = scrpool.tile([97, SBLK], bf16, name=f"drb{blk}", tag="drb")
                for j in range(NSUB):
                    nc.vector.tensor_scalar_add(drf[:, ts(j, 128)],
                                                den_all[0:97, ts(j, 128)], EPS)
                    nc.vector.reciprocal(drf[:, ts(j, 128)], drf[:, ts(j, 128)])
                    nc.vector.tensor_copy(drb[:, ts(j, 128)], drf[:, ts(j, 128)])

                # broadcast recip across partitions (per-piece), stage to SBUF
                bc_ps = []
                for h in range(H_PER):
                    bc_p = psmisc.tile([128, SBLK], f32, name=f"bcp{blk}_{h}", tag="m")
                    bc_ps.append(bc_p)
                for j in range(NSUB):
                    for h in range(H_PER):
                        nc.tensor.matmul(bc_ps[h][:, ts(j, 128)],
                                         ones_row[32 * h:32 * h + 1, :],
                                         drb[32 * h:32 * h + 1, ts(j, 128)],
                                         start=(j == 0), stop=(j == NSUB - 1),
                                         tile_position=(32 * h, 0),
                                         skip_group_check=True)
                bcb = []
                for h in range(H_PER):
                    bb = bcbpool.tile([128, SBLK], bf16, name=f"bcb{blk}_{h}", tag="bcb")
                    for c2 in range(NCH):
                        nc.scalar.copy(bb[:, ts(c2, CW)], bc_ps[h][:, ts(c2, CW)])
                    bcb.append(bb)

                # numerators in short-lived [128,256] chunks, divided
                outT = []
                for h in range(H_PER):
                    oT = otpool.tile([128, SBLK], bf16, name=f"oT{blk}_{h}", tag="outT")
                    for c2 in range(NCH):
                        lo, hi = CW * c2, CW * (c2 + 1)
                        nt = psmisc.tile([128, CW], f32, name=f"nt{blk}_{h}_{c2}", tag="m")
                        first = True
                        for i2 in range(NSUB):
                            st = max(lo, 128 * i2)
                            if st >= hi:
                                break
                            nc.tensor.matmul(
                                nt[:, st - lo:hi - lo], vha[:, i2, h, 0:128],
                                atm[(i2, h)][:, st - 128 * i2:hi - 128 * i2],
                                start=first, stop=False, skip_group_check=True)
                            first = False
                        nc.tensor.matmul(nt[:], Zb16[32 * h:32 * h + NB, 0:128],
                                         qfe[32 * h:32 * h + NB, lo:hi],
                                         start=False, stop=True,
                                         tile_position=(32 * h, 0),
                                         skip_group_check=True)
                        nc.vector.tensor_mul(oT[:, lo:hi], nt[:], bcb[h][:, lo:hi])
                    outT.append(oT)

                # state update: all 4 heads into one bank (col-group packed)
                su_p = psmisc.tile([128, SBLK], f32, name=f"su{blk}", tag="m")
                nc.vector.memset(su_p[:], 0.0)
                for i2 in range(NSUB):
                    for h in range(H_PER):
                        nc.tensor.matmul(su_p[32 * h:32 * h + NB, 0:129],
                                         kfn[:, i2, 32 * h:32 * h + NB],
                                         vha[:, i2, h, 0:129],
                                         start=False,
                                         stop=(i2 == NSUB - 1 and h == H_PER - 1),
                                         tile_position=(0, 32 * h),
                                         skip_group_check=True)
                for h in range(H_PER):
                    nc.vector.tensor_add(Zsb[32 * h:32 * h + NB, 0:129],
                                         Zsb[32 * h:32 * h + NB, 0:129],
                                         su_p[32 * h:32 * h + NB, 0:129])
                for h in range(H_PER):
                    nc.vector.tensor_copy(Zb16[32 * h:32 * h + NB, 0:129],
                                          Zsb[32 * h:32 * h + NB, 0:129])

                # output projection
                for j in range(NSUB):
                    osb = osbpool.tile([128, D], bf16, name=f"osb{blk}_{j}", tag="osb")
                    for c in range(4):
                        op = psmisc.tile([128, 512], f32, name=f"op{blk}_{j}_{c}", tag="m")
                        for h in range(H_PER):
                            nc.tensor.matmul(op[:], outT[h][:, ts(j, 128)],
                                             wo_sb[:, h, ts(c, 512)],
                                             start=(h == 0), stop=(h == H_PER - 1))
                        nc.any.tensor_copy(osb[:, ts(c, 512)], op[:])
                    r0 = s0 + 128 * j
                    nc.sync.dma_start(part_d[r0:r0 + 128, :], osb[:])

            def warmup(n, name):
                # zero-dependency PE work: flips the HAM clock gate warm
                # during the startup DMA wait and fills the pipeline-prime
                # window; writes scratch PSUM that is never read.
                dp = psmisc.tile([128, 128], f32, name=name, tag="m")
                for i in range(n):
                    nc.tensor.matmul(dp[:], ident[:], ident[:],
                                     start=True, stop=True,
                                     skip_group_check=True)

            # software pipeline: scan(k-1) issued before proj(k)
            warmup(40, "wu0")
            prev = None
            for k in range(NBLK + 1):
                if k >= 1:
                    scan_oproj(k - 1, prev)
                if k < NBLK:
                    prev = proj(k)
                if k == 1:
                    warmup(64, "wu1")

    nc.compile()
    return nc


def _pad_feat(w):
    """[4, 8, D] head-feature weights -> [D, 128] with head h at cols 32h."""
    out = np.zeros((128, D), np.float32)
    for h in range(H_PER):
        out[32 * h:32 * h + NB] = w[h]
    return np.ascontiguousarray(out.T)


def _prep_inputs(q, k, v, w_q, w_k, w_v, w_o, omega):
    """Host-side sharding: returns in_maps for the 8 cores."""
    bf = ml_dtypes.bfloat16
    mask = np.ones((128, SBLK), np.float32)
    mask[:, :128] = np.triu(np.ones((128, 128), np.float32))

    xs = []
    for b in range(B):
        xs.append((np.ascontiguousarray(q[b].T).astype(bf),
                   np.ascontiguousarray(k[b].T).astype(bf),
                   np.ascontiguousarray(v[b].T).astype(bf)))

    wq_h = w_q.reshape(16, DK, D)
    wk_h = w_k.reshape(16, DK, D)
    wqom = np.einsum('nd,hde->hne', omega, wq_h)
    wkom = np.einsum('nd,hde->hne', omega, wk_h)

    in_maps = []
    for core in range(8):
        b, g = divmod(core, 4)
        sl = slice(512 * g, 512 * (g + 1))
        hsl = slice(4 * g, 4 * (g + 1))
        xq, xk, xv = xs[b]
        in_maps.append({
            "xq": xq, "xk": xk, "xv": xv,
            "wqom": _pad_feat(wqom[hsl]).astype(bf),
            "wkom": _pad_feat(wkom[hsl]).astype(bf),
            "wv": np.ascontiguousarray(w_v[sl, :].T).astype(bf),
            "wo": np.ascontiguousarray(w_o[:, sl].T).astype(bf),
            "mask": mask,
        })
    return in_maps


def kernel(q, k, v, w_q, w_k, w_v, w_o, omega):
    global LAST_EXEC_TIME_NS
    q, k, v = np.asarray(q), np.asarray(k), np.asarray(v)
    w_q, w_k, w_v, w_o = (np.asarray(a) for a in (w_q, w_k, w_v, w_o))
    omega = np.asarray(omega)

    if "nc" not in _CACHE:
        _CACHE["nc"] = _build()
    nc = _CACHE["nc"]

    in_maps = _prep_inputs(q, k, v, w_q, w_k, w_v, w_o, omega)
    trace = bool(os.environ.get("BASS_KERNEL_TRACE"))
    res = run_bass_kernel_spmd(nc, in_maps, core_ids=list(range(8)), trace=trace)
    LAST_EXEC_TIME_NS = res.exec_time_ns

    out = np.zeros((B, S, D), np.float32)
    for core in range(8):
        b = core // 4
        out[b] += res.results[core]["part"].astype(np.float32)
    return out


# revision 36
# speedup vs baseline: 1.0055x; 1.0055x over previous
"""Causal Performer (FAVOR+) Trainium2 kernel.

Sharding: 8 cores = 2 (batch) x 4 (head groups of 4 heads).  Each core
computes its 4 heads for one batch and returns a partial [4096, 2048]
output (its heads' contribution through w_o) in bf16; the host sums
the 4 partials per batch in f32.

Structure: software-pipelined over 512-position sequence blocks.  Each
iteration issues SCAN(k-1) (sparse, DVE-gated small matmuls) BEFORE
PROJ(k) (dense N=512 matmuls with no compute deps), so the scheduler
fills every scan stall with projection work and the PE activity never
dips low enough for the HAM clock gate to re-throttle.

  - Packed-head layout: the 4 heads' 8 random features live at
    partition offsets 32h of one [128, 512] tile; exp/transpose/reduce/
    normalize are whole-tile ops, and the small matmuls use 32-aligned
    tile_position packing (den: M=1 col groups, su: M=8 col groups,
    A^T/Z: K=8 row groups).
  - Numerator accumulated in short-lived [128,256] PSUM chunks per
    (head, half), divided by the broadcast reciprocal as produced.
  - den/su accumulate interleaved per-head chains into memset-zeroed
    banks with start=False (a start=True whole-bank has_written clear
    would corrupt sibling heads).
  - Output partials in bf16; host sums in f32.
"""

import os
import numpy as np
import ml_dtypes

from concourse import bacc, mybir
import concourse.tile as tile
from concourse.bass import ts
from concourse.bass_utils import run_bass_kernel_spmd
from concourse.masks import make_identity

B, S, D = 2, 4096, 2048
H_PER = 4            # heads per core
DK = 128
NB = 8
SBLK = 512           # sequence block
NBLK = S // SBLK     # 8
NSUB = SBLK // 128   # 4 sub-chunks of 128
NCH = 2              # numerator chunks per head (width 256)
CW = SBLK // NCH     # 256
EPS = 1e-6

bf16 = mybir.dt.bfloat16
f32 = mybir.dt.float32

LAST_EXEC_TIME_NS = None
_CACHE = {}


def _build():
    nc = bacc.Bacc("TRN2", target_bir_lowering=False, debug=False)

    xq_d = nc.dram_tensor("xq", [D, S], bf16, kind="ExternalInput").ap()
    xk_d = nc.dram_tensor("xk", [D, S], bf16, kind="ExternalInput").ap()
    xv_d = nc.dram_tensor("xv", [D, S], bf16, kind="ExternalInput").ap()
    wqom_d = nc.dram_tensor("wqom", [D, 128], bf16, kind="ExternalInput").ap()
    wkom_d = nc.dram_tensor("wkom", [D, 128], bf16, kind="ExternalInput").ap()
    wv_d = nc.dram_tensor("wv", [D, 512], bf16, kind="ExternalInput").ap()
    wo_d = nc.dram_tensor("wo", [512, D], bf16, kind="ExternalInput").ap()
    mask_d = nc.dram_tensor("mask", [128, SBLK], f32, kind="ExternalInput").ap()
    part_d = nc.dram_tensor("part", [S, D], bf16, kind="ExternalOutput").ap()

    KC = D // 128
    Exp = mybir.ActivationFunctionType.Exp

    with tile.TileContext(nc) as tc:
        with tc.tile_pool(name="const", bufs=1) as const, \
             tc.tile_pool(name="wpool", bufs=1) as wpool, \
             tc.tile_pool(name="state", bufs=1) as state, \
             tc.tile_pool(name="xpool", bufs=2) as xpool, \
             tc.tile_pool(name="vpool", bufs=2) as vpool, \
             tc.tile_pool(name="featpool", bufs=2) as featpool, \
             tc.tile_pool(name="atmpool", bufs=1) as atmpool, \
             tc.tile_pool(name="otpool", bufs=6) as otpool, \
             tc.tile_pool(name="bcbpool", bufs=4) as bcbpool, \
             tc.tile_pool(name="osbpool", bufs=2) as osbpool, \
             tc.tile_pool(name="miscpool", bufs=2) as miscpool, \
             tc.tile_pool(name="scrpool", bufs=1) as scrpool, \
             tc.tile_pool(name="psqk", bufs=2, space="PSUM") as psqk, \
             tc.tile_pool(name="psv", bufs=2, space="PSUM") as psv, \
             tc.tile_pool(name="psden", bufs=1, space="PSUM") as psden, \
             tc.tile_pool(name="psmisc", bufs=3, space="PSUM") as psmisc:

            ident = const.tile([128, 128], bf16, name="ident")
            make_identity(nc, ident)
            mask_sb = const.tile([128, SBLK], f32, name="mask_sb")
            nc.sync.dma_start(mask_sb[:], mask_d[:])
            ones_col = const.tile([128, 1], bf16, name="ones_col")
            nc.vector.memset(ones_col[:], 1.0)
            ones_row = const.tile([97, 128], bf16, name="ones_row")
            nc.vector.memset(ones_row[:], 1.0)

            wqom_sb = wpool.tile([128, KC, 128], bf16, name="wqom_sb")
            nc.sync.dma_start(wqom_sb[:], wqom_d.rearrange("(c p) m -> p c m", p=128))
            wkom_sb = wpool.tile([128, KC, 128], bf16, name="wkom_sb")
            nc.sync.dma_start(wkom_sb[:], wkom_d.rearrange("(c p) m -> p c m", p=128))
            # wv/wo DMAs issued after block 0's x loads (cold-start order)
            wv_sb = wpool.tile([128, KC, 512], bf16, name="wv_sb")
            wo_sb = wpool.tile([128, H_PER, D], bf16, name="wo_sb")

            # persistent scan state, head h at partitions 32h..32h+8:
            # cols 0:128 = Z, col 128 = z
            Zsb = state.tile([128, 132], f32, name="Zsb")
            nc.vector.memset(Zsb[:], 0.0)
            Zb16 = state.tile([128, 132], bf16, name="Zb16")
            nc.vector.memset(Zb16[:], 0.0)

            def proj(blk):
                """Projections for block blk: returns tiles dict."""
                s0 = blk * SBLK
                xq_sb = xpool.tile([128, KC, SBLK], bf16, name=f"xq{blk}", tag="xq")
                nc.sync.dma_start(
                    xq_sb[:],
                    xq_d.rearrange("(c p) s -> p c s", p=128)[:, :, s0:s0 + SBLK])
                xk_sb = xpool.tile([128, KC, SBLK], bf16, name=f"xk{blk}", tag="xk")
                nc.sync.dma_start(
                    xk_sb[:],
                    xk_d.rearrange("(c p) s -> p c s", p=128)[:, :, s0:s0 + SBLK])
                xv_sb = xpool.tile([128, KC, SBLK], bf16, name=f"xv{blk}", tag="xv")
                nc.sync.dma_start(
                    xv_sb[:],
                    xv_d.rearrange("(c p) s -> p c s", p=128)[:, :, s0:s0 + SBLK])
                if blk == 0:
                    nc.sync.dma_start(wv_sb[:], wv_d.rearrange("(c p) m -> p c m", p=128))
                    nc.sync.dma_start(wo_sb[:], wo_d.rearrange("(c p) m -> p c m", p=128))

                # fused q/k feature projections: head h rows at 32h
                qf_p = psqk.tile([128, SBLK], f32, name=f"qfp{blk}", tag="qk")
                kf_p = psqk.tile([128, SBLK], f32, name=f"kfp{blk}", tag="qk")
                for dst, wsb, xsb in ((qf_p, wqom_sb, xq_sb), (kf_p, wkom_sb, xk_sb)):
                    for kc in range(KC):
                        nc.tensor.matmul(dst[:], wsb[:, kc, :], xsb[:, kc, :],
                                         start=(kc == 0), stop=(kc == KC - 1))
                qsq = scrpool.tile([128, SBLK], f32, name=f"qsq{blk}", tag="qsq")
                nc.scalar.square(qsq[:], qf_p[:])
                ksq = scrpool.tile([128, SBLK], f32, name=f"ksq{blk}", tag="ksq")
                nc.scalar.square(ksq[:], kf_p[:])
                qfe = featpool.tile([128, SBLK], bf16, name=f"qfe{blk}", tag="qfe")
                nc.scalar.activation(qfe[:], qsq[:], Exp, scale=-0.5)
                kfe = featpool.tile([128, SBLK], bf16, name=f"kfe{blk}", tag="kfe")
                nc.scalar.activation(kfe[:], ksq[:], Exp, scale=-0.5)

                # k features transposed to [s, f-packed]
                kfs = featpool.tile([128, NSUB, 128], bf16, name=f"kfs{blk}", tag="kfs")
                for j in range(NSUB):
                    kT_p = psmisc.tile([128, 128], bf16, name=f"kT{blk}_{j}", tag="m")
                    nc.tensor.transpose(kT_p[:], kfe[:, ts(j, 128)], ident[:])
                    nc.vector.tensor_copy(kfs[:, j, :], kT_p[:])

                # normalizers: ksum[s, j*4+h] over the 8 real features
                ksum = miscpool.tile([128, NSUB * H_PER], f32, name=f"ksum{blk}", tag="ksum")
                nc.vector.reduce_sum(
                    ksum[:].rearrange("p (a b) -> p a b", b=H_PER),
                    kfs[:].rearrange("p a (b c) -> p a b c", c=32)[:, :, :, 0:NB],
                    axis=mybir.AxisListType.X)
                nc.vector.tensor_scalar_add(ksum[:], ksum[:], EPS)
                krec = miscpool.tile([128, NSUB * H_PER], f32, name=f"krec{blk}", tag="krec")
                nc.vector.reciprocal(krec[:], ksum[:])

                # normalized k features for the state update
                kfn = featpool.tile([128, NSUB, 128], bf16, name=f"kfn{blk}", tag="kfn")
                for j in range(NSUB):
                    for h in range(H_PER):
                        nc.vector.tensor_scalar(
                            out=kfn[:, j, 32 * h:32 * h + 32],
                            in0=kfs[:, j, 32 * h:32 * h + 32],
                            scalar1=krec[:, 4 * j + h:4 * j + h + 1], scalar2=None,
                            op0=mybir.AluOpType.mult)

                # v projection: vha [s_sub(128), j, head, 132] (+ones col)
                vha = vpool.tile([128, NSUB, H_PER, 132], bf16, name=f"vha{blk}", tag="vha")
                for j in range(NSUB):
                    pp = psv.tile([128, SBLK], f32, name=f"pv{blk}_{j}", tag="pp")
                    for kc in range(KC):
                        nc.tensor.matmul(pp[:], xv_sb[:, kc, ts(j, 128)],
                                         wv_sb[:, kc, :],
                                         start=(kc == 0), stop=(kc == KC - 1))
                    nc.scalar.copy(vha[:, j, :, 0:128],
                                   pp.rearrange("p (h d) -> p h d", d=128))
                    nc.vector.memset(vha[:, j, :, 128:129], 1.0)

                return dict(qfe=qfe, kfe=kfe, kfn=kfn, krec=krec, vha=vha)

            def scan_oproj(blk, t):
                """Scan + output projection for block blk using tiles t."""
                s0 = blk * SBLK
                qfe, kfe, kfn, krec, vha = (t[k] for k in
                                            ("qfe", "kfe", "kfn", "krec", "vha"))

                # masked A^T blocks (row-group packed), scale+mask on DVE
                atm = {}
                for i2 in range(NSUB):
                    n_i = SBLK - 128 * i2
                    atp = []
                    for h in range(H_PER):
                        at_p = psmisc.tile([128, SBLK], f32, name=f"at{blk}_{i2}_{h}", tag="m")
                        nc.tensor.matmul(at_p[:, :n_i], kfe[32 * h:32 * h + NB, ts(i2, 128)],
                                         qfe[32 * h:32 * h + NB, 128 * i2:SBLK],
                                         start=True, stop=True,
                                         tile_position=(32 * h, 0))
                        atp.append(at_p)
                    for h in range(H_PER):
                        am = atmpool.tile([128, n_i], bf16, name=f"am{blk}_{i2}_{h}",
                                          tag=f"atm{i2}", bufs=4)
                        nc.vector.scalar_tensor_tensor(
                            out=am[:], in0=atp[h][:, :n_i],
                            scalar=krec[:, 4 * i2 + h:4 * i2 + h + 1],
                            in1=mask_sb[:, :n_i],
                            op0=mybir.AluOpType.mult, op1=mybir.AluOpType.mult)
                        atm[(i2, h)] = am

                # denominators, head h at partition 32h (col-group packed);
                # memset + start=False: interleaved per-head chains must
                # accumulate onto zeroed data (start would clear bank-wide).
                den_all = psden.tile([128, SBLK], f32, name=f"den{blk}", tag="den")
                nc.vector.memset(den_all[:], 0.0)
                for i2 in range(NSUB):
                    for h in range(H_PER):
                        nc.tensor.matmul(den_all[32 * h:32 * h + 1, 128 * i2:SBLK],
                                         ones_col[:], atm[(i2, h)][:],
                                         start=False, stop=False,
                                         tile_position=(0, 32 * h),
                                         skip_group_check=True)
                for h in range(H_PER):
                    nc.tensor.matmul(den_all[32 * h:32 * h + 1, :],
                                     Zb16[32 * h:32 * h + NB, 128:129],
                                     qfe[32 * h:32 * h + NB, :],
                                     start=False, stop=(h == H_PER - 1),
                                     tile_position=(32 * h, 32 * h),
                                     skip_group_check=True)

                # reciprocal of denominators, pipelined in 128-col pieces
                # (a single [97,512] reciprocal is ~3.4us and head-of-line
                # blocks the bc matmuls in the PE stream)
                drf = scrpool.tile([97, SBLK], f32, name=f"drf{blk}", tag="drf")
                drb = scrpool.tile([97, SBLK], bf16, name=f"drb{blk}", tag="drb")
                for j in range(NSUB):
                    nc.vector.tensor_scalar_add(drf[:, ts(j, 128)],
                                                den_all[0:97, ts(j, 128)], EPS)
                    nc.vector.reciprocal(drf[:, ts(j, 128)], drf[:, ts(j, 128)])
                    nc.vector.tensor_copy(drb[:, ts(j, 128)], drf[:, ts(j, 128)])

                # broadcast recip across partitions (per-piece), stage to SBUF
                bc_ps = []
                for h in range(H_PER):
                    bc_p = psmisc.tile([128, SBLK], f32, name=f"bcp{blk}_{h}", tag="m")
                    bc_ps.append(bc_p)
                for j in range(NSUB):
                    for h in range(H_PER):
                        nc.tensor.matmul(bc_ps[h][:, ts(j, 128)],
                                         ones_row[32 * h:32 * h + 1, :],
                                         drb[32 * h:32 * h + 1, ts(j, 128)],
                                         start=(j == 0), stop=(j == NSUB - 1),
                                         tile_position=(32 * h, 0),
                                         skip_group_check=True)
                bcb = []
                for h in range(H_PER):
                    bb = bcbpool.tile([128, SBLK], bf16, name=f"bcb{blk}_{h}", tag="bcb")
                    for c2 in range(NCH):
                        nc.scalar.copy(bb[:, ts(c2, CW)], bc_ps[h][:, ts(c2, CW)])
                    bcb.append(bb)

                # numerators in short-lived [128,256] chunks, divided
                outT = []
                for h in range(H_PER):
                    oT = otpool.tile([128, SBLK], bf16, name=f"oT{blk}_{h}", tag="outT")
                    for c2 in range(NCH):
                        lo, hi = CW * c2, CW * (c2 + 1)
                        nt = psmisc.tile([128, CW], f32, name=f"nt{blk}_{h}_{c2}", tag="m")
                        first = True
                        for i2 in range(NSUB):
                            st = max(lo, 128 * i2)
                            if st >= hi:
                                break
                            nc.tensor.matmul(
                                nt[:, st - lo:hi - lo], vha[:, i2, h, 0:128],
                                atm[(i2, h)][:, st - 128 * i2:hi - 128 * i2],
                                start=first, stop=False, skip_group_check=True)
                            first = False
                        nc.tensor.matmul(nt[:], Zb16[32 * h:32 * h + NB, 0:128],
                                         qfe[32 * h:32 * h + NB, lo:hi],
                                         start=False, stop=True,
                                         tile_position=(32 * h, 0),
                                         skip_group_check=True)
                        nc.vector.tensor_mul(oT[:, lo:hi], nt[:], bcb[h][:, lo:hi])
                    outT.append(oT)

                # state update: all 4 heads into one bank (col-group packed)
                su_p = psmisc.tile([128, SBLK], f32, name=f"su{blk}", tag="m")
                nc.vector.memset(su_p[:], 0.0)
                for i2 in range(NSUB):
                    for h in range(H_PER):
                        nc.tensor.matmul(su_p[32 * h:32 * h + NB, 0:129],
                                         kfn[:, i2, 32 * h:32 * h + NB],
                                         vha[:, i2, h, 0:129],
                                         start=False,
                                         stop=(i2 == NSUB - 1 and h == H_PER - 1),
                                         tile_position=(0, 32 * h),
                                         skip_group_check=True)
                for h in range(H_PER):
                    nc.vector.tensor_add(Zsb[32 * h:32 * h + NB, 0:129],
                                         Zsb[32 * h:32 * h + NB, 0:129],
                                         su_p[32 * h:32 * h + NB, 0:129])
                for h in range(H_PER):
                    nc.vector.tensor_copy(Zb16[32 * h:32 * h + NB, 0:129],
                                          Zsb[32 * h:32 * h + NB, 0:129])

                # output projection
                for j in range(NSUB):
                    osb = osbpool.tile([128, D], bf16, name=f"osb{blk}_{j}", tag="osb")
                    for c in range(4):
                        op = psmisc.tile([128, 512], f32, name=f"op{blk}_{j}_{c}", tag="m")
                        for h in range(H_PER):
                            nc.tensor.matmul(op[:], outT[h][:, ts(j, 128)],
                                             wo_sb[:, h, ts(c, 512)],
                                             start=(h == 0), stop=(h == H_PER - 1))
                        nc.any.tensor_copy(osb[:, ts(c, 512)], op[:])
                    r0 = s0 + 128 * j
                    nc.sync.dma_start(part_d[r0:r0 + 128, :], osb[:])

            # software pipeline: scan(k-1) issued before proj(k)
            prev = None
            for k in range(NBLK + 1):
                if k >= 1:
                    scan_oproj(k - 1, prev)
                if k < NBLK:
                    prev = proj(k)

    nc.compile()
    return nc


def _pad_feat(w):
    """[4, 8, D] head-feature weights -> [D, 128] with head h at cols 32h."""
    out = np.zeros((128, D), np.float32)
    for h in range(H_PER):
        out[32 * h:32 * h + NB] = w[h]
    return np.ascontiguousarray(out.T)


def _prep_inputs(q, k, v, w_q, w_k, w_v, w_o, omega):
    """Host-side sharding: returns in_maps for the 8 cores."""
    bf = ml_dtypes.bfloat16
    mask = np.ones((128, SBLK), np.float32)
    mask[:, :128] = np.triu(np.ones((128, 128), np.float32))

    xs = []
    for b in range(B):
        xs.append((np.ascontiguousarray(q[b].T).astype(bf),
                   np.ascontiguousarray(k[b].T).astype(bf),
                   np.ascontiguousarray(v[b].T).astype(bf)))

    wq_h = w_q.reshape(16, DK, D)
    wk_h = w_k.reshape(16, DK, D)
    wqom = np.einsum('nd,hde->hne', omega, wq_h)
    wkom = np.einsum('nd,hde->hne', omega, wk_h)

    in_maps = []
    for core in range(8):
        b, g = divmod(core, 4)
        sl = slice(512 * g, 512 * (g + 1))
        hsl = slice(4 * g, 4 * (g + 1))
        xq, xk, xv = xs[b]
        in_maps.append({
            "xq": xq, "xk": xk, "xv": xv,
            "wqom": _pad_feat(wqom[hsl]).astype(bf),
            "wkom": _pad_feat(wkom[hsl]).astype(bf),
            "wv": np.ascontiguousarray(w_v[sl, :].T).astype(bf),
            "wo": np.ascontiguousarray(w_o[:, sl].T).astype(bf),
            "mask": mask,
        })
    return in_maps


def kernel(q, k, v, w_q, w_k, w_v, w_o, omega):
    global LAST_EXEC_TIME_NS
    q, k, v = np.asarray(q), np.asarray(k), np.asarray(v)
    w_q, w_k, w_v, w_o = (np.asarray(a) for a in (w_q, w_k, w_v, w_o))
    omega = np.asarray(omega)

    if "nc" not in _CACHE:
        _CACHE["nc"] = _build()
    nc = _CACHE["nc"]

    in_maps = _prep_inputs(q, k, v, w_q, w_k, w_v, w_o, omega)
    trace = bool(os.environ.get("BASS_KERNEL_TRACE"))
    res = run_bass_kernel_spmd(nc, in_maps, core_ids=list(range(8)), trace=trace)
    LAST_EXEC_TIME_NS = res.exec_time_ns

    out = np.zeros((B, S, D), np.float32)
    for core in range(8):
        b = core // 4
        out[b] += res.results[core]["part"].astype(np.float32)
    return out
